# revision 1
# baseline (speedup 1.0000x reference)
"""GNN decoder (nn_Decoder) Trainium2 kernel — 8-core SPMD, data-parallel.

Contract: kernel(**inputs) takes the FULL unsharded inputs (same keys as
reference.setup_inputs()) and returns the full (atom_type, atom_charge,
bond_order) tuple, computed on 8 NeuronCores via Bass/Tile +
run_bass_kernel_spmd.

Sharding: nodes and edges are split evenly across the 8 cores (data
parallel); the small MLP weights are replicated.  Each core also gets a
replicated copy of the full `quantized` table in its HBM so the per-edge
gathers quantized[src] / quantized[dst] are local indirect DMAs — no
cross-core collectives are needed.

Per-core kernel (Tile framework):
  * atom MLP: activations are kept transposed ([feat, rows]) so each
    matmul contracts over partitions with weights in natural [K, M] layout.
    The host pre-transposes the node slice (input layout prep).
  * bond MLP: rows of `quantized` are gathered with single-index-per-
    partition indirect DMAs (128 rows each), summed (src+dst) on DVE,
    transposed to [feat, edge] layout on the tensor engine, and the RBF
    distance expansion is computed on-chip (DVE + ACT) from host-gathered
    endpoint coordinates.  The 544-feature input then runs the 3-layer MLP.
  * matmuls use float32r (fp32 storage, full-rate PE) — measured ~3e-4
    absmax relative error end-to-end vs the fp32 reference.

Outputs land transposed ([out_feat, rows]); the host transposes back and
strips the padding.
"""

from dataclasses import dataclass

import numpy as np

import concourse.bass as bass
import concourse.mybir as mybir
from concourse import bacc
from concourse.bass import IndirectOffsetOnAxis
from concourse.bass_utils import run_bass_kernel_spmd
from concourse.masks import make_identity
from concourse.tile import TileContext

F32 = mybir.dt.float32
F32R = mybir.dt.float32r
I32 = mybir.dt.int32
AF = mybir.ActivationFunctionType

N_NODES = 50000
N_EDGES = 100000
RBF_DMAX = 10.0
RBF_DIM = 32
SIGMA = RBF_DMAX / RBF_DIM
INV_SIG2 = 1.0 / (SIGMA * SIGMA)
EPS = 1e-8


@dataclass
class Cfg:
    table_rows: int = N_NODES  # rows of the replicated gather table
    latent: int = 512
    hid: int = 1024
    nodes_pc: int = 6400       # padded nodes per core (multiple of 256)
    edges_pc: int = 12544      # padded edges per core (multiple of 128)
    aout: int = 16
    bout: int = 5
    n_cores: int = 8
    mm_f32r: bool = True       # use float32r fast matmul mode


def build(cfg: Cfg):
    L, H = cfg.latent, cfg.hid
    KL = L // 128              # latent K-chunks (4)
    KH = H // 128              # hid K-chunks (8)
    MH = H // 128              # hid M-chunks (8)
    n_echunks = cfg.edges_pc // 128

    MD = F32R if cfg.mm_f32r else F32

    nc = bacc.Bacc("TRN2", target_bir_lowering=False)

    def wdma(dst, src):
        # f32 -> f32r needs a casting DMA, which only SWDGE supports
        if cfg.mm_f32r:
            nc.gpsimd.dma_start(dst, src)
        else:
            nc.sync.dma_start(dst, src)

    # ---- DRAM I/O ----
    qT = nc.dram_tensor("qT", [L, cfg.nodes_pc], F32, kind="ExternalInput")
    qfull = nc.dram_tensor("qfull", [cfg.table_rows, L], F32, kind="ExternalInput")
    isrc = nc.dram_tensor("isrc", [128, n_echunks], I32, kind="ExternalInput")
    idst = nc.dram_tensor("idst", [128, n_echunks], I32, kind="ExternalInput")
    xsrc = nc.dram_tensor("xsrc", [128, n_echunks * 3], F32, kind="ExternalInput")
    xdst = nc.dram_tensor("xdst", [128, n_echunks * 3], F32, kind="ExternalInput")
    wa0 = nc.dram_tensor("wa0", [L, H], F32, kind="ExternalInput")
    wa1 = nc.dram_tensor("wa1", [H, H], F32, kind="ExternalInput")
    wa2 = nc.dram_tensor("wa2", [H, cfg.aout], F32, kind="ExternalInput")
    wb0 = nc.dram_tensor("wb0", [L + RBF_DIM, H], F32, kind="ExternalInput")
    wb1 = nc.dram_tensor("wb1", [H, H], F32, kind="ExternalInput")
    wb2 = nc.dram_tensor("wb2", [H, cfg.bout], F32, kind="ExternalInput")
    ba0 = nc.dram_tensor("ba0", [128, MH], F32, kind="ExternalInput")
    ba1 = nc.dram_tensor("ba1", [128, MH], F32, kind="ExternalInput")
    ba2 = nc.dram_tensor("ba2", [cfg.aout, 1], F32, kind="ExternalInput")
    bb0 = nc.dram_tensor("bb0", [128, MH], F32, kind="ExternalInput")
    bb1 = nc.dram_tensor("bb1", [128, MH], F32, kind="ExternalInput")
    bb2 = nc.dram_tensor("bb2", [cfg.bout, 1], F32, kind="ExternalInput")
    muq = nc.dram_tensor("muq", [128, RBF_DIM], F32, kind="ExternalInput")
    aoutT = nc.dram_tensor("aoutT", [cfg.aout, cfg.nodes_pc], F32,
                           kind="ExternalOutput")
    boutT = nc.dram_tensor("boutT", [cfg.bout, cfg.edges_pc], F32,
                           kind="ExternalOutput")

    # column tiles of up to 512 (groups of 128)
    def col_tiles(total):
        tiles = []
        c = 0
        while c * 128 < total:
            g = min(4, total // 128 - c)
            tiles.append((c, g))
            c += g
        return tiles

    atom_tiles = col_tiles(cfg.nodes_pc)
    edge_tiles = col_tiles(cfg.edges_pc)

    with TileContext(nc) as tc:
        with tc.tile_pool(name="const", bufs=1) as constp:
            ident = constp.tile([128, 128], F32)
            make_identity(nc, ident[:])
            muq_sb = constp.tile([128, RBF_DIM], F32)
            nc.sync.dma_start(muq_sb[:], muq[:])
            ba0_sb = constp.tile([128, MH], F32)
            nc.sync.dma_start(ba0_sb[:], ba0[:])
            ba1_sb = constp.tile([128, MH], F32)
            nc.sync.dma_start(ba1_sb[:], ba1[:])
            ba2_sb = constp.tile([cfg.aout, 1], F32)
            nc.sync.dma_start(ba2_sb[:], ba2[:])
            bb0_sb = constp.tile([128, MH], F32)
            nc.sync.dma_start(bb0_sb[:], bb0[:])
            bb1_sb = constp.tile([128, MH], F32)
            nc.sync.dma_start(bb1_sb[:], bb1[:])
            bb2_sb = constp.tile([cfg.bout, 1], F32)
            nc.sync.dma_start(bb2_sb[:], bb2[:])
            isrc_sb = constp.tile([128, n_echunks], I32)
            nc.sync.dma_start(isrc_sb[:], isrc[:])
            idst_sb = constp.tile([128, n_echunks], I32)
            nc.sync.dma_start(idst_sb[:], idst[:])
            xs_sb = constp.tile([128, n_echunks, 3], F32)
            nc.sync.dma_start(xs_sb[:], xsrc[:].rearrange("p (c k) -> p c k", k=3))
            xd_sb = constp.tile([128, n_echunks, 3], F32)
            nc.sync.dma_start(xd_sb[:], xdst[:].rearrange("p (c k) -> p c k", k=3))
            eps_sb = constp.tile([128, 1], F32)
            nc.vector.memset(eps_sb[:], EPS * INV_SIG2)

            # ================= ATOM PHASE =================
            with (
                tc.tile_pool(name="aw", bufs=1) as awp,
                tc.tile_pool(name="aact", bufs=2) as aact,
                tc.tile_pool(name="apsum", bufs=2, space="PSUM") as apsum,
            ):
                wa0_sb = awp.tile([128, KL, H], MD)
                wdma(wa0_sb[:], wa0[:].rearrange("(c p) m -> p c m", p=128))
                wa1_sb = awp.tile([128, KH, H], MD)
                wdma(wa1_sb[:], wa1[:].rearrange("(c p) m -> p c m", p=128))
                wa2_sb = awp.tile([128, KH, cfg.aout], MD)
                wdma(wa2_sb[:], wa2[:].rearrange("(c p) m -> p c m", p=128))

                for (c0, G) in atom_tiles:
                    N = G * 128
                    n0 = c0 * 128
                    qt = aact.tile([128, KL, N], MD, tag="qt")
                    wdma(
                        qt[:], qT[:, n0:n0 + N].rearrange("(c p) n -> p c n", p=128))
                    h0 = aact.tile([128, MH, N], MD, tag="ah0")
                    for m in range(MH):
                        ps = apsum.tile([128, N], F32, tag="amm")
                        for c in range(KL):
                            nc.tensor.matmul(
                                ps[:], wa0_sb[:, c, m * 128:(m + 1) * 128],
                                qt[:, c, :], start=(c == 0), stop=(c == KL - 1))
                        nc.scalar.activation(h0[:, m, :], ps[:], AF.Relu,
                                             bias=ba0_sb[:, m:m + 1])
                    h1 = aact.tile([128, MH, N], MD, tag="ah1")
                    for m in range(MH):
                        ps = apsum.tile([128, N], F32, tag="amm")
                        for c in range(KH):
                            nc.tensor.matmul(
                                ps[:], wa1_sb[:, c, m * 128:(m + 1) * 128],
                                h0[:, c, :], start=(c == 0), stop=(c == KH - 1))
                        nc.scalar.activation(h1[:, m, :], ps[:], AF.Relu,
                                             bias=ba1_sb[:, m:m + 1])
                    ps2 = apsum.tile([cfg.aout, N], F32, tag="aout")
                    for c in range(KH):
                        nc.tensor.matmul(ps2[:], wa2_sb[:, c, :], h1[:, c, :],
                                         start=(c == 0), stop=(c == KH - 1))
                    ao = aact.tile([cfg.aout, N], F32, tag="ao")
                    nc.vector.tensor_scalar_add(ao[:], ps2[:], ba2_sb[:, 0:1])
                    nc.sync.dma_start(aoutT[:, n0:n0 + N], ao[:])

            # ================= BOND PHASE =================
            with (
                tc.tile_pool(name="bw", bufs=1) as bwp,
                tc.tile_pool(name="bact", bufs=2) as bact,
                tc.tile_pool(name="bgs", bufs=8) as bgs,
                tc.tile_pool(name="bgd", bufs=4) as bgd,
                tc.tile_pool(name="bh", bufs=1) as bh,
                tc.tile_pool(name="bpsum", bufs=2, space="PSUM") as bpsum,
                tc.tile_pool(name="btp", bufs=3, space="PSUM") as btp,
            ):
                wb0_sb = bwp.tile([128, KL, H], MD)
                wdma(wb0_sb[:], wb0[0:L, :].rearrange("(c p) m -> p c m", p=128))
                wb0r_sb = bwp.tile([RBF_DIM, H], MD)
                wdma(wb0r_sb[:], wb0[L:L + RBF_DIM, :])
                wb1_sb = bwp.tile([128, KH, H], MD)
                wdma(wb1_sb[:], wb1[:].rearrange("(c p) m -> p c m", p=128))
                wb2_sb = bwp.tile([128, KH, cfg.bout], MD)
                wdma(wb2_sb[:], wb2[:].rearrange("(c p) m -> p c m", p=128))

                for (c0, G) in edge_tiles:
                    N = G * 128
                    e0 = c0 * 128
                    gss = []
                    for g in range(G):
                        a = bgs.tile([128, L], F32, tag="gs")
                        nc.gpsimd.indirect_dma_start(
                            out=a[:], out_offset=None, in_=qfull[:],
                            in_offset=IndirectOffsetOnAxis(
                                ap=isrc_sb[:, c0 + g:c0 + g + 1], axis=0))
                        b = bgd.tile([128, L], F32, tag="gd")
                        nc.gpsimd.indirect_dma_start(
                            out=b[:], out_offset=None, in_=qfull[:],
                            in_offset=IndirectOffsetOnAxis(
                                ap=idst_sb[:, c0 + g:c0 + g + 1], axis=0))
                        nc.vector.tensor_add(a[:], a[:], b[:])
                        gss.append(a)

                    # RBF distances
                    dx = bact.tile([128, G, 3], F32, tag="dx")
                    nc.vector.tensor_sub(dx[:], xs_sb[:, c0:c0 + G, :],
                                         xd_sb[:, c0:c0 + G, :])
                    nc.vector.tensor_mul(dx[:], dx[:], dx[:])
                    d2 = bact.tile([128, G, 1], F32, tag="d2")
                    nc.vector.reduce_sum(d2[:], dx[:], axis=mybir.AxisListType.X)
                    dsc = bact.tile([128, G, 1], F32, tag="dsc")
                    # dsc = sqrt((d2 + EPS) * INV_SIG2) = d / sigma
                    nc.scalar.activation(dsc[:], d2[:], AF.Sqrt,
                                         scale=INV_SIG2, bias=eps_sb[:])
                    rbf = bact.tile([128, G, RBF_DIM], F32, tag="rbf")
                    for g in range(G):
                        z = bact.tile([128, RBF_DIM], F32, tag="z")
                        nc.vector.tensor_scalar(z[:], muq_sb[:], dsc[:, g, :], None,
                                                op0=mybir.AluOpType.subtract)
                        nc.vector.tensor_mul(z[:], z[:], z[:])
                        nc.scalar.activation(rbf[:, g, :], z[:], AF.Exp, scale=-1.0)

                    # transpose gathered features to [feat, edge]
                    qsT = bact.tile([128, KL, N], MD, tag="qsT")
                    for f in range(KL):
                        pt = btp.tile([128, N], F32, tag="pt")
                        for g in range(G):
                            nc.tensor.transpose(
                                pt[:, g * 128:(g + 1) * 128],
                                gss[g][:, f * 128:(f + 1) * 128], ident[:])
                        nc.vector.tensor_copy(qsT[:, f, :], pt[:])
                    rbfT = bact.tile([RBF_DIM, N], MD, tag="rbfT")
                    pt5 = btp.tile([128, N], F32, tag="pt")
                    for g in range(G):
                        nc.tensor.transpose(pt5[:RBF_DIM, g * 128:(g + 1) * 128],
                                            rbf[:, g, :], ident[:])
                    nc.vector.tensor_copy(rbfT[:], pt5[:RBF_DIM, :])

                    h0 = bh.tile([128, MH, N], MD, tag="bh0")
                    for m in range(MH):
                        ps = bpsum.tile([128, N], F32, tag="bmm")
                        for c in range(KL):
                            nc.tensor.matmul(
                                ps[:], wb0_sb[:, c, m * 128:(m + 1) * 128],
                                qsT[:, c, :], start=(c == 0), stop=False)
                        nc.tensor.matmul(ps[:], wb0r_sb[:, m * 128:(m + 1) * 128],
                                         rbfT[:], start=False, stop=True)
                        nc.scalar.activation(h0[:, m, :], ps[:], AF.Relu,
                                             bias=bb0_sb[:, m:m + 1])
                    h1 = bh.tile([128, MH, N], MD, tag="bh1")
                    for m in range(MH):
                        ps = bpsum.tile([128, N], F32, tag="bmm")
                        for c in range(KH):
                            nc.tensor.matmul(
                                ps[:], wb1_sb[:, c, m * 128:(m + 1) * 128],
                                h0[:, c, :], start=(c == 0), stop=(c == KH - 1))
                        nc.scalar.activation(h1[:, m, :], ps[:], AF.Relu,
                                             bias=bb1_sb[:, m:m + 1])
                    ps2 = bpsum.tile([cfg.bout, N], F32, tag="bout")
                    for c in range(KH):
                        nc.tensor.matmul(ps2[:], wb2_sb[:, c, :], h1[:, c, :],
                                         start=(c == 0), stop=(c == KH - 1))
                    bo = bact.tile([cfg.bout, N], F32, tag="bo")
                    nc.vector.tensor_scalar_add(bo[:], ps2[:], bb2_sb[:, 0:1])
                    nc.sync.dma_start(boutT[:, e0:e0 + N], bo[:])

    nc.finalize()
    return nc


def _interleave_cols(a: np.ndarray) -> np.ndarray:
    """[n*128, ...] -> [128, n, ...] with (p, c) = row c*128+p."""
    n = a.shape[0] // 128
    return np.ascontiguousarray(
        a.reshape(n, 128, *a.shape[1:]).transpose(1, 0, *range(2, a.ndim + 1)))


def prep_in_maps(inputs: dict, cfg: Cfg):
    q = np.ascontiguousarray(np.asarray(inputs["quantized"], dtype=np.float32))
    x = np.asarray(inputs["x"], dtype=np.float32)
    pair = np.asarray(inputs["pair_indices"]).astype(np.int32)
    n_nodes = q.shape[0]
    n_edges = pair.shape[0]
    nodes_pc_real = n_nodes // cfg.n_cores
    edges_pc_real = n_edges // cfg.n_cores

    mu = np.linspace(0.0, RBF_DMAX, RBF_DIM, dtype=np.float32)
    muq_arr = np.ascontiguousarray(
        np.broadcast_to(mu / SIGMA, (128, RBF_DIM)).astype(np.float32))

    def chunked_bias(b):
        return np.ascontiguousarray(
            np.asarray(b, np.float32).reshape(cfg.hid // 128, 128).T)

    common = {
        "qfull": q,
        "wa0": np.asarray(inputs["atom_w0"], np.float32),
        "wa1": np.asarray(inputs["atom_w1"], np.float32),
        "wa2": np.asarray(inputs["atom_w2"], np.float32),
        "wb0": np.asarray(inputs["bond_w0"], np.float32),
        "wb1": np.asarray(inputs["bond_w1"], np.float32),
        "wb2": np.asarray(inputs["bond_w2"], np.float32),
        "ba0": chunked_bias(inputs["atom_b0"]),
        "ba1": chunked_bias(inputs["atom_b1"]),
        "ba2": np.ascontiguousarray(
            np.asarray(inputs["atom_b2"], np.float32).reshape(cfg.aout, 1)),
        "bb0": chunked_bias(inputs["bond_b0"]),
        "bb1": chunked_bias(inputs["bond_b1"]),
        "bb2": np.ascontiguousarray(
            np.asarray(inputs["bond_b2"], np.float32).reshape(cfg.bout, 1)),
        "muq": muq_arr,
    }

    in_maps = []
    for c in range(cfg.n_cores):
        nslice = q[c * nodes_pc_real:(c + 1) * nodes_pc_real]
        qT_c = np.zeros((cfg.latent, cfg.nodes_pc), np.float32)
        qT_c[:, :nodes_pc_real] = nslice.T
        epair = pair[c * edges_pc_real:(c + 1) * edges_pc_real]
        pad = cfg.edges_pc - edges_pc_real
        src = np.concatenate([epair[:, 0], np.zeros(pad, np.int32)])
        dst = np.concatenate([epair[:, 1], np.zeros(pad, np.int32)])
        m = {
            "qT": qT_c,
            "isrc": _interleave_cols(src),
            "idst": _interleave_cols(dst),
            "xsrc": _interleave_cols(x[src]).reshape(128, -1),
            "xdst": _interleave_cols(x[dst]).reshape(128, -1),
        }
        m.update(common)
        in_maps.append(m)
    return in_maps


def post_outputs(results, cfg: Cfg, n_nodes: int, n_edges: int):
    nodes_pc_real = n_nodes // cfg.n_cores
    edges_pc_real = n_edges // cfg.n_cores
    atom = np.concatenate(
        [results[c]["aoutT"].T[:nodes_pc_real] for c in range(cfg.n_cores)], axis=0)
    bond = np.concatenate(
        [results[c]["boutT"].T[:edges_pc_real] for c in range(cfg.n_cores)], axis=0)
    return (np.ascontiguousarray(atom[:, :10]),
            np.ascontiguousarray(atom[:, 10:16]),
            np.ascontiguousarray(bond))


_NC_CACHE: dict = {}


def _get_nc(cfg: Cfg):
    key = (cfg.nodes_pc, cfg.edges_pc, cfg.mm_f32r)
    if key not in _NC_CACHE:
        _NC_CACHE[key] = build(cfg)
    return _NC_CACHE[key]


def kernel(**inputs) -> tuple:
    cfg = Cfg()
    nc = _get_nc(cfg)
    in_maps = prep_in_maps(inputs, cfg)
    res = run_bass_kernel_spmd(nc, in_maps, core_ids=list(range(cfg.n_cores)))
    return post_outputs(res.results, cfg, N_NODES, N_EDGES)


if __name__ == "__main__":
    rng = np.random.default_rng(0)
    inputs = {
        "x": rng.standard_normal((N_NODES, 3), dtype=np.float32) * 5,
        "quantized": rng.standard_normal((N_NODES, 512), dtype=np.float32),
        "pair_indices": rng.integers(0, N_NODES, (N_EDGES, 2)).astype(np.int64),
        "atom_w0": rng.standard_normal((512, 1024), dtype=np.float32) / 512 ** 0.5,
        "atom_b0": np.zeros(1024, np.float32),
        "atom_w1": rng.standard_normal((1024, 1024), dtype=np.float32) / 32,
        "atom_b1": np.zeros(1024, np.float32),
        "atom_w2": rng.standard_normal((1024, 16), dtype=np.float32) / 32,
        "atom_b2": np.zeros(16, np.float32),
        "bond_w0": rng.standard_normal((544, 1024), dtype=np.float32) / 544 ** 0.5,
        "bond_b0": np.zeros(1024, np.float32),
        "bond_w1": rng.standard_normal((1024, 1024), dtype=np.float32) / 32,
        "bond_b1": np.zeros(1024, np.float32),
        "bond_w2": rng.standard_normal((1024, 5), dtype=np.float32) / 32,
        "bond_b2": np.zeros(5, np.float32),
    }
    outs = kernel(**inputs)
    print([o.shape for o in outs])
